# revision 17
# baseline (speedup 1.0000x reference)
"""Trainium2 Bass kernel for nn_MLA_KVSplice (MLA attention with KVSplice
latent bottleneck), tensor-parallel over heads across 8 NeuronCores.

Strategy (per core c, heads {2c, 2c+1}):
  - All big tensors live/compute in transposed layouts so every matmul
    contraction sits on the partition dim.
  - Every DRAM input is pre-laid on the host in its exact SBUF layout
    ([P, free...] contiguous) so each DMA is 128 contiguous descriptors.
  - Startup: wklT + xT chunk0 stream in interleaved quarters on the sync
    queue; the PE starts on the first quarter (~6us) instead of waiting
    for all weights.  All other weights load on the gpsimd queue behind a
    blocker op so they don't steal HBM bandwidth from the critical path.
  - Latent pipeline (x->kv_latent->splice->latent_p) is replicated per core;
    LN's affine (g,b) is folded into We/bfrom on the host, normalization uses
    a rank-1 correction applied after the expand matmul.  1/std and the
    softmax 1/L use the fast approximate DVE reciprocal (~5x cheaper).
  - K/V of chunk i-1 are emitted inside chunk i's latent matmul stream so
    the PE never stalls on the DVE latent-correction chain.
  - Per-head causal attention in S^T[j,i] layout: exp without max-subtraction
    (|scores/sqrt(hd)| < ~1.5), row-sums via ones-matmul on the PE, fully
    masked j-tiles skipped.  ps_l / ps_o live in separate psum rings so the
    next iteration never waits on the epilogue.
  - Row-parallel out-proj staggered into the attention loop (out-proj of
    i-chunk ic-1 runs between attention iterations); psum copyback
    alternates between the scalar and vector engines.
  - Each core emits a fp16 partial [T, D]; host sums the 8 partials in
    fp32 and adds bout.
"""

import math
import os

import numpy as np

import concourse.bass as bass
import concourse.tile as tile
from concourse import bacc, mybir
from concourse.bass_utils import run_bass_kernel_spmd

# problem constants (hardcoded per harness contract)
B, T, D = 1, 2048, 2048
H, HD = 16, 128
DLAT, DCMP = 512, 256
THETA = 10000.0
LN_EPS = 1e-5
N_CORES = 8
HPC = H // N_CORES          # heads per core = 2
M = HPC * HD                # per-core head dims = 256

P = 128                     # partitions
TCH = 512                   # t-chunk for pass 1
NT = T // TCH               # 4
NKO = D // P                # 16 contraction chunks over model dim
NLC = DLAT // P             # 4
NCC = DCMP // P             # 2
NIC = T // 512              # 4 i-chunks in attention
NJC = T // P                # 16 j-chunks
NTC16 = T // P              # 16 row-chunks in out-proj

F16 = mybir.dt.float16
F32 = mybir.dt.float32
AF = mybir.ActivationFunctionType
ALU = mybir.AluOpType

ATT_SCALE = 1.0 / math.sqrt(HD)

_CACHE = {}

LAST_RESULT = None  # BassKernelResults of the most recent run (for test.py)


def _recip(nc, out, in_):
    """1/in_ on DVE; fast approx when available (18 bits, plenty here)."""
    if hasattr(nc.vector, "reciprocal_approx_fast"):
        nc.vector.reciprocal_approx_fast(out=out[:], in_=in_[:])
    else:
        nc.vector.reciprocal(out[:], in_[:])


def _build():
    if "nc" in _CACHE:
        return _CACHE["nc"]

    nc = bacc.Bacc(None, target_bir_lowering=False)

    def din(name, shape, dt):
        return nc.dram_tensor(name, shape, dt, kind="ExternalInput")

    # every input is pre-laid host-side in its exact SBUF layout
    xTp_d = din("xTp", [NT, P, NKO * TCH], F16)
    wklp_d = din("wklp", [P, NKO * DLAT], F16)
    wqp_d = din("wqp", [P, NKO, M], F16)
    wcp_d = din("wcp", [P, NLC, DCMP], F16)
    wegp_d = din("wegp", [P, NCC, DLAT], F16)
    wfkp_d = din("wfkp", [P, NLC, M], F16)
    wfvp_d = din("wfvp", [P, NLC, M], F16)
    woutp_d = din("woutp", [P, HPC, D], F16)
    cosp_d = din("cosp", [P, T], F16)
    sinp_d = din("sinp", [P, T], F16)
    maskp_d = din("maskp", [P, 4, 512], F16)
    onesp_d = din("onesp", [P, P], F16)
    # all small per-partition vectors in ONE tensor -> one DMA
    # cols: 0:4 lat_scale | 4:8 lat_bias | 8:12 negs | 12:14 bq | 14:16 bk
    vecs_d = din("vecs", [P, 17], F32)
    bvp_d = din("bvp", [P, M], F16)

    out_d = nc.dram_tensor("out_partial", [NTC16, P, D], F16,
                           kind="ExternalOutput")

    with tile.TileContext(nc) as tc:
        with (
            tc.tile_pool(name="consts", bufs=1) as cp,
            tc.tile_pool(name="persist", bufs=1) as pp,
            tc.tile_pool(name="work", bufs=2) as wp,
            tc.tile_pool(name="psum", bufs=4, space="PSUM") as psp,
        ):
            # ---- tiny consts first on the gpsimd queue ----
            vecs = cp.tile([P, 17], F32, tag="vecs", name="vecs")
            nc.gpsimd.dma_start(vecs[:], vecs_d[:, :])
            ones16 = cp.tile([P, P], F16, tag="ones16", name="ones16")
            nc.gpsimd.dma_start(ones16[:], onesp_d[:, :])
            maskT = cp.tile([P, 4, 512], F16, tag="maskT", name="maskT")
            nc.gpsimd.dma_start(maskT[:], maskp_d[:, :, :])

            lat_scale = vecs[:, 0:NLC]
            lat_bias = vecs[:, NLC:2 * NLC]
            negs = vecs[:, 2 * NLC:3 * NLC]
            bq2 = vecs[:, 12:12 + HPC]
            bk2 = vecs[:, 14:14 + HPC]
            epsc = vecs[:, 16:17]

            # ---- critical-path loads: wklT and xT chunk0, in pieces ----
            NPC = 8                  # pieces
            KOP = NKO // NPC         # ko's per piece
            wklT = cp.tile([P, NKO, DLAT], F16, tag="wklT", name="wklT")
            xT_t0 = wp.tile([P, NKO, TCH], F16, tag="xT", name="xT_t0")
            wkl_r = wklp_d[:, :].rearrange("p (ko l) -> p ko l", l=DLAT)
            x0_r = xTp_d[0, :, :].rearrange("p (ko t) -> p ko t", t=TCH)
            for kp in range(NPC):
                ks = slice(kp * KOP, (kp + 1) * KOP)
                nc.sync.dma_start(wklT[:, ks, :], wkl_r[:, ks, :])
                nc.sync.dma_start(xT_t0[:, ks, :], x0_r[:, ks, :])

            # rope tables + compress weights ride the sync queue right behind
            # the startup pieces (the gpsimd queue would deliver them late)
            cosT = cp.tile([P, T], F16, tag="cosT", name="cosT")
            nc.sync.dma_start(cosT[:], cosp_d[:, :])
            sinT = cp.tile([P, T], F16, tag="sinT", name="sinT")
            nc.sync.dma_start(sinT[:], sinp_d[:, :])
            wcT = cp.tile([P, NLC, DCMP], F16, tag="wcT", name="wcT")
            nc.sync.dma_start(wcT[:], wcp_d[:, :, :])

            # blocker: the remaining const loads (gpsimd queue) wait for most
            # of the critical startup stream so they don't steal HBM
            # bandwidth from it
            blk = cp.tile([P, 1], F16, tag="blk", name="blk")
            nc.gpsimd.tensor_copy(blk[:], xT_t0[:, NKO * 3 // 4 - 1, 0:1])

            def gload(name, dram_ap, shape):
                t = cp.tile(shape, F16, tag=name, name=name)
                nc.gpsimd.dma_start(t[:], dram_ap)
                return t

            wegT = gload("wegT", wegp_d[:, :, :], [P, NCC, DLAT])
            wqT = gload("wqT", wqp_d[:, :, :], [P, NKO, M])
            wfkT = gload("wfkT", wfkp_d[:, :, :], [P, NLC, M])
            wfvT = gload("wfvT", wfvp_d[:, :, :], [P, NLC, M])
            bv_bc = gload("bv_bc", bvp_d[:, :], [P, M])
            woutT = gload("woutT", woutp_d[:, :, :], [P, HPC, D])

            # ---- persistent intermediates ----
            kT_all = pp.tile([P, HPC, T], F16, tag="kT_all", name="kT_all")
            qT_all = pp.tile([P, HPC, T], F16, tag="qT_all", name="qT_all")
            v_all = pp.tile([P, NJC, M], F16, tag="v_all", name="v_all")
            outT_all = pp.tile([P, HPC, T], F16, tag="outT_all",
                               name="outT_all")

            # ======== pass 1: projections, splice, K/V (sw-pipelined) ======
            # state carried from chunk i-1 into chunk i's emission
            prev = {}

            HHD = HD // 2

            def emit_rope(pre, dst_slice, tsl):
                """dst = pre*cos + perm(pre)*sin' with the half-rotation done
                as an SBUF->SBUF partition swap on the (idle) DMA engines and
                the rotation sign folded into sinT host-side."""
                perm = wp.tile([P, TCH], F16, tag="perm", name="perm", bufs=4)
                nc.gpsimd.dma_start(perm[0:HHD, :], pre[HHD:P, :])
                nc.gpsimd.dma_start(perm[HHD:P, :], pre[0:HHD, :])
                t1 = wp.tile([P, TCH], F16, tag="ropet1", name="rope_t1")
                t2 = wp.tile([P, TCH], F16, tag="ropet2", name="rope_t2")
                nc.vector.tensor_mul(t1[:], pre[:], cosT[:, tsl])
                nc.vector.tensor_mul(t2[:], perm[:], sinT[:, tsl])
                nc.vector.tensor_add(dst_slice, t1[:], t2[:])

            def emit_K(pv):
                tsl = pv["tsl"]
                for h in range(HPC):
                    psk = psp.tile([P, TCH], F32, tag="ps", name="ps_k")
                    for lc in range(NLC):
                        nc.tensor.matmul(
                            psk[:],
                            wfkT[:, lc, h * HD:(h + 1) * HD],
                            pv["latpT"][:, lc, :],
                            start=(lc == 0), stop=(lc == NLC - 1),
                        )
                    kpre = wp.tile([P, TCH], F16, tag="kpre", name="kpre")
                    nc.scalar.activation(kpre[:], psk[:], AF.Identity,
                                         bias=bk2[:, h:h + 1])
                    emit_rope(kpre, kT_all[:, h, tsl], tsl)

            def emit_V(pv):
                tci = pv["tci"]
                for ts4 in range(TCH // P):
                    psv = psp.tile([P, TCH], F32, tag="ps", name="ps_v")
                    for lc in range(NLC):
                        nc.tensor.matmul(
                            psv[:, 0:M],
                            pv["latpT"][:, lc, ts4 * P:(ts4 + 1) * P],
                            wfvT[:, lc, :],
                            start=(lc == 0), stop=(lc == NLC - 1),
                        )
                    nc.vector.tensor_add(v_all[:, tci * (TCH // P) + ts4, :],
                                         psv[:, 0:M], bv_bc[:])

            for tci in range(NT):
                tsl = slice(tci * TCH, (tci + 1) * TCH)
                if tci == 0:
                    xT_t = xT_t0
                else:
                    xT_t = wp.tile([P, NKO, TCH], F16, tag="xT", name="xT_t")
                    xi_r = xTp_d[tci, :, :].rearrange("p (ko t) -> p ko t",
                                                      t=TCH)
                    nc.sync.dma_start(xT_t[:], xi_r[:, :, :])

                xtT_t = wp.tile([P, NLC, TCH], F16, tag="xtT", name="xtT_t",
                                bufs=1)

                # --- kv latent (transposed) + KVSplice input scaling ---
                if tci == 0:
                    # ko-outer: consume the startup DMA quarters as they land
                    ps4 = [psp.tile([P, TCH], F32, tag="ps", name=f"ps_l{lc}")
                           for lc in range(NLC)]
                    for ko in range(NKO):
                        for lc in range(NLC):
                            nc.tensor.matmul(
                                ps4[lc][:],
                                wklT[:, ko, lc * P:(lc + 1) * P],
                                xT_t[:, ko, :],
                                start=(ko == 0), stop=(ko == NKO - 1),
                                skip_group_check=True,
                            )
                    for lc in range(NLC):
                        nc.scalar.activation(
                            xtT_t[:, lc], ps4[lc][:], AF.Identity,
                            bias=lat_bias[:, lc:lc + 1],
                            scale=lat_scale[:, lc:lc + 1],
                        )
                else:
                    # lc-outer, with chunk i-1's K and V interleaved so the
                    # PE never waits on the DVE latent-correction chain
                    for lc in range(NLC):
                        ps = psp.tile([P, TCH], F32, tag="ps", name="ps_lat")
                        for ko in range(NKO):
                            nc.tensor.matmul(
                                ps[:],
                                wklT[:, ko, lc * P:(lc + 1) * P],
                                xT_t[:, ko, :],
                                start=(ko == 0), stop=(ko == NKO - 1),
                            )
                        nc.scalar.activation(
                            xtT_t[:, lc], ps[:], AF.Identity,
                            bias=lat_bias[:, lc:lc + 1],
                            scale=lat_scale[:, lc:lc + 1],
                        )
                        if lc == 1 and prev:
                            emit_K(prev)
                        if lc == 3 and prev:
                            emit_V(prev)

                # --- compress y^T = Wc @ xt^T  [DCMP, t] ---
                yT_t = wp.tile([P, NCC, TCH], F16, tag="yT", name="yT_t",
                               bufs=1)
                ysq_t = wp.tile([P, NCC, TCH], F16, tag="ysq", name="ysq_t",
                                bufs=1)
                for cc in range(NCC):
                    psy = psp.tile([P, TCH], F32, tag="ps", name="ps_y")
                    for lc in range(NLC):
                        nc.tensor.matmul(
                            psy[:],
                            wcT[:, lc, cc * P:(cc + 1) * P],
                            xtT_t[:, lc, :],
                            start=(lc == 0), stop=(lc == NLC - 1),
                        )
                    nc.scalar.copy(yT_t[:, cc], psy[:])
                    nc.scalar.square(ysq_t[:, cc], psy[:])

                # --- LN stats over DCMP via ones-matmul (bcast to 128p) ---
                ps1 = psp.tile([P, TCH], F32, tag="acA", name="ps_s1", bufs=2)
                ps2 = psp.tile([P, TCH], F32, tag="acB", name="ps_s2", bufs=2)
                for cc in range(NCC):
                    nc.tensor.matmul(ps1[:], ones16[:], yT_t[:, cc],
                                     start=(cc == 0), stop=(cc == NCC - 1),
                                     skip_group_check=True)
                for cc in range(NCC):
                    nc.tensor.matmul(ps2[:], ones16[:], ysq_t[:, cc],
                                     start=(cc == 0), stop=(cc == NCC - 1),
                                     skip_group_check=True)
                mu = wp.tile([P, TCH], F32, tag="mu", name="mu", bufs=1)
                musq = wp.tile([P, TCH], F32, tag="musq", name="musq", bufs=1)
                m2 = wp.tile([P, TCH], F32, tag="m2", name="m2", bufs=1)
                std = wp.tile([P, TCH], F32, tag="std", name="std", bufs=1)
                At = wp.tile([P, TCH], F32, tag="At", name="At", bufs=1)
                Bt = wp.tile([P, TCH], F32, tag="Bt", name="Bt", bufs=1)
                nc.scalar.mul(mu[:], ps1[:], 1.0 / DCMP)
                # m2 = E[y^2] + eps  (eps folded into the copyback bias)
                nc.scalar.activation(m2[:], ps2[:], AF.Identity,
                                     bias=epsc[:, 0:1], scale=1.0 / DCMP)
                nc.scalar.square(musq[:], mu[:])
                nc.vector.tensor_sub(m2[:], m2[:], musq[:])
                nc.scalar.sqrt(std[:], m2[:])
                _recip(nc, At, std)
                nc.vector.tensor_mul(Bt[:], mu[:], At[:])

                # --- Q (+bias, rope): independent filler over the LN chain --
                for hc in range(HPC):
                    psq = psp.tile([P, TCH], F32, tag="ps", name="ps_q")
                    for ko in range(NKO):
                        nc.tensor.matmul(
                            psq[:],
                            wqT[:, ko, hc * HD:(hc + 1) * HD],
                            xT_t[:, ko, :],
                            start=(ko == 0), stop=(ko == NKO - 1),
                        )
                    qpre = wp.tile([P, TCH], F16, tag="qpre", name="qpre")
                    nc.scalar.activation(qpre[:], psq[:], AF.Identity,
                                         bias=bq2[:, hc:hc + 1])
                    emit_rope(qpre, qT_all[:, hc, tsl], tsl)

                # --- expand latent_p^T = We_g @ z^T (rank-1 LN correction) ---
                latpT_t = wp.tile([P, NLC, TCH], F16, tag="latpT",
                                  name="latpT_t")
                for lc in range(NLC):
                    pse = psp.tile([P, TCH], F32, tag="ps", name="ps_e")
                    for cc in range(NCC):
                        nc.tensor.matmul(
                            pse[:],
                            wegT[:, cc, lc * P:(lc + 1) * P],
                            yT_t[:, cc, :],
                            start=(cc == 0), stop=(cc == NCC - 1),
                        )
                    etmp = wp.tile([P, TCH], F32, tag="etmp", name="etmp")
                    nc.vector.tensor_mul(etmp[:], pse[:], At[:])
                    nc.vector.scalar_tensor_tensor(
                        latpT_t[:, lc], Bt[:], negs[:, lc:lc + 1], etmp[:],
                        op0=ALU.mult, op1=ALU.add,
                    )

                prev = {"tci": tci, "tsl": tsl, "latpT": latpT_t}

            # last chunk's K/V
            emit_K(prev)
            emit_V(prev)

            # ====== pass 2 + staggered pass 3 (attention + out-proj) ======
            def emit_outproj(ic):
                for tt in range(4 * ic, 4 * ic + 4):
                    out_sb = wp.tile([P, D // 512, 512], F16, tag="outsb",
                                     name="out_sb")
                    for dc in range(D // 512):
                        psp_o = psp.tile([P, 512], F32, tag="ps", name="ps_P")
                        for hc in range(HPC):
                            nc.tensor.matmul(
                                psp_o[:],
                                outT_all[:, hc, tt * P:(tt + 1) * P],
                                woutT[:, hc, dc * 512:(dc + 1) * 512],
                                start=(hc == 0), stop=(hc == HPC - 1),
                            )
                        # DVE copyback: the scalar engine is saturated by the
                        # attention exps that run concurrently
                        nc.vector.tensor_copy(out_sb[:, dc], psp_o[:])
                    nc.sync.dma_start(out_d[tt, :, :],
                                      out_sb[:].rearrange("p a b -> p (a b)"))

            for ic in range(NIC):
                isl = slice(ic * 512, (ic + 1) * 512)
                njc = 4 * ic + 4
                for h in range(HPC):
                    ps_l = psp.tile([P, 512], F32, tag="acA", name="ps_L",
                                    bufs=2)
                    ps_o = psp.tile([P, 512], F32, tag="acB", name="ps_O",
                                    bufs=2)
                    WIN = 4
                    Pts = {}

                    def lv(jc, ps_l=ps_l, ps_o=ps_o, njc=njc, h=h, Pts=Pts):
                        Pt = Pts.pop(jc)
                        nc.tensor.matmul(ps_l[:], ones16[:], Pt[:],
                                         start=(jc == 0), stop=(jc == njc - 1),
                                         skip_group_check=True)
                        nc.tensor.matmul(ps_o[:],
                                         v_all[:, jc, h * HD:(h + 1) * HD],
                                         Pt[:],
                                         start=(jc == 0), stop=(jc == njc - 1),
                                         skip_group_check=True)

                    for jc in range(njc):
                        ps_s = psp.tile([P, 512], F32, tag="ps", name="ps_S")
                        nc.tensor.matmul(
                            ps_s[:],
                            kT_all[:, h, jc * P:(jc + 1) * P],
                            qT_all[:, h, isl],
                            start=True, stop=True,
                        )
                        dd = jc - 4 * ic
                        if dd >= 0:
                            nc.vector.tensor_add(ps_s[:], ps_s[:],
                                                 maskT[:, dd])
                        Pt = wp.tile([P, 512], F16, tag="P", name="P_t",
                                     bufs=6)
                        nc.scalar.activation(Pt[:], ps_s[:], AF.Exp,
                                             scale=ATT_SCALE)
                        Pts[jc] = Pt
                        if jc >= WIN:
                            lv(jc - WIN)
                    for jc in range(max(0, njc - WIN), njc):
                        lv(jc)
                    Linv = wp.tile([P, 512], F32, tag="Linv", name="Linv")
                    _recip(nc, Linv, ps_l)
                    nc.vector.tensor_mul(outT_all[:, h, isl], ps_o[:],
                                         Linv[:])
                # out-proj of the previous i-chunk: its inputs are long
                # ready, so it fills the PE while this ic's epilogue drains
                if ic > 0:
                    emit_outproj(ic - 1)
            emit_outproj(NIC - 1)

    nc.compile()  # bacc passes: split multi-waits into event semaphores etc.
    _CACHE["nc"] = nc
    return nc


def _host_prep(x, Wq, bq, Wkl, bkl, t_scale, t_shift, Wc, We, ln_g, ln_b,
               Wfrom, bfrom, Wout, bout):
    """Build the 8 per-core input maps (shard + transpose + fold on host).

    Every array is laid out exactly as its SBUF tile ([P, free...]) so the
    DMA is 128 contiguous per-partition descriptors."""
    f16 = np.float16
    f32 = np.float32

    x2 = np.ascontiguousarray(x.reshape(T, D))
    xT = np.ascontiguousarray(x2.T).astype(f16)          # [D, T]
    # xTp[tci, p, ko*TCH + t] = xT[ko*P + p, tci*TCH + t]
    xTp = np.ascontiguousarray(
        xT.reshape(NKO, P, NT, TCH).transpose(2, 1, 0, 3).reshape(
            NT, P, NKO * TCH))

    sp = np.log1p(np.exp(t_scale.astype(np.float64))).astype(f32)  # softplus
    lat_scale = np.ascontiguousarray(sp.reshape(NLC, P).T).astype(f32)
    lat_bias_v = (bkl * sp + t_shift).astype(f32)
    lat_bias = np.ascontiguousarray(lat_bias_v.reshape(NLC, P).T).astype(f32)

    We_g = (We * ln_g[None, :]).astype(f32)
    bias_exp = (We_g @ ln_b).astype(f32)
    s_vec = (We_g @ np.ones(DCMP, f32)).astype(f32)
    negs = np.ascontiguousarray((-s_vec).reshape(NLC, P).T).astype(f32)

    def sb_layout(wT, nko, free):
        # wT: [K, free] with K = nko*P -> [P, nko, free]
        return np.ascontiguousarray(
            wT.reshape(nko, P, free).transpose(1, 0, 2)).astype(f16)

    wklp = sb_layout(np.ascontiguousarray(Wkl.T), NKO, DLAT).reshape(
        P, NKO * DLAT)
    wcp = sb_layout(np.ascontiguousarray(Wc.T), NLC, DCMP)
    wegp = sb_layout(np.ascontiguousarray(We_g.T), NCC, DLAT)

    # rope tables
    t_idx = np.arange(T, dtype=f32)
    inv_freq = 1.0 / THETA ** (np.arange(0, HD, 2, dtype=f32) / HD)
    freqs = t_idx[:, None] * inv_freq[None, :]
    cosp = np.ascontiguousarray(
        np.concatenate([np.cos(freqs), np.cos(freqs)], axis=1).T).astype(f16)
    # rotation sign folded in: rot(x)[p] = -x[p+64] (p<64), +x[p-64] (p>=64)
    sinp = np.ascontiguousarray(
        np.concatenate([-np.sin(freqs), np.sin(freqs)], axis=1).T).astype(f16)

    onesp = np.ones((P, P), f16)

    # additive causal masks for the 4 diagonal deltas (j0 = i0 + 128*d)
    maskp = np.zeros((P, 4, 512), f16)
    pidx = np.arange(P)[:, None]
    fidx = np.arange(512)[None, :]
    for d in range(4):
        maskp[:, d, :] = np.where(pidx <= fidx - 128 * d, 0.0, -30000.0)

    shared = dict(
        xTp=xTp, wklp=wklp, wcp=wcp, wegp=wegp, cosp=cosp, sinp=sinp,
        maskp=maskp, onesp=onesp,
    )

    WfK = Wfrom[:H * HD]
    WfV = Wfrom[H * HD:]
    bfK = bfrom[:H * HD]
    bfV = bfrom[H * HD:]

    in_maps = []
    for c in range(N_CORES):
        hsl = slice(c * M, (c + 1) * M)
        wqp = sb_layout(np.ascontiguousarray(Wq[hsl].T), NKO, M)
        bq_c = np.ascontiguousarray(bq[hsl].reshape(HPC, P).T).astype(f32)
        WfK_c = WfK[hsl]
        WfV_c = WfV[hsl]
        wfkp = sb_layout(np.ascontiguousarray(WfK_c.T), NLC, M)
        wfvp = sb_layout(np.ascontiguousarray(WfV_c.T), NLC, M)
        bk_eff = (bfK[hsl] + WfK_c @ bias_exp).astype(f32)
        bv_eff = (bfV[hsl] + WfV_c @ bias_exp).astype(f32)
        bk2 = np.ascontiguousarray(bk_eff.reshape(HPC, P).T).astype(f32)
        bvp = np.broadcast_to(bv_eff.astype(f16)[None, :], (P, M)).copy()
        woutp = sb_layout(np.ascontiguousarray(Wout[:, hsl].T), HPC, D)
        vecs = np.zeros((P, 17), f32)
        vecs[:, 0:NLC] = lat_scale
        vecs[:, NLC:2 * NLC] = lat_bias
        vecs[:, 2 * NLC:3 * NLC] = negs
        vecs[:, 12:12 + HPC] = bq_c
        vecs[:, 14:14 + HPC] = bk2
        vecs[:, 16] = LN_EPS
        in_maps.append(dict(
            shared,
            wqp=wqp, wfkp=wfkp, wfvp=wfvp, vecs=vecs,
            bvp=bvp, woutp=woutp,
        ))
    return in_maps


def kernel(x, Wq, bq, Wkl, bkl, t_scale, t_shift, Wc, We, ln_g, ln_b,
           Wfrom, bfrom, Wout, bout):
    global LAST_RESULT
    args = dict(x=x, Wq=Wq, bq=bq, Wkl=Wkl, bkl=bkl, t_scale=t_scale,
                t_shift=t_shift, Wc=Wc, We=We, ln_g=ln_g, ln_b=ln_b,
                Wfrom=Wfrom, bfrom=bfrom, Wout=Wout, bout=bout)
    args = {k: np.asarray(v, dtype=np.float32) for k, v in args.items()}

    nc = _build()
    in_maps = _host_prep(**args)

    want_trace = bool(int(os.environ.get("BASS_TRACE", "0") or "0"))
    try:
        res = run_bass_kernel_spmd(
            nc, in_maps, core_ids=list(range(N_CORES)), trace=want_trace,
        )
    except ModuleNotFoundError:
        os.environ["BASS_NEVER_TRACE"] = "1"
        res = run_bass_kernel_spmd(
            nc, in_maps, core_ids=list(range(N_CORES)), trace=False,
        )
    LAST_RESULT = res

    acc = np.zeros((T, D), np.float32)
    for r in res.results:
        acc += r["out_partial"].reshape(T, D).astype(np.float32)
    acc += args["bout"][None, :]
    return acc[None].astype(np.float32)


if __name__ == "__main__":
    rng = np.random.default_rng(0)
    # smoke test with random inputs (not the reference distribution)
    import reference as ref
    import jax
    with jax.default_device(jax.devices("cpu")[0]):
        inputs = {k: np.asarray(v, np.float32)
                  for k, v in ref.setup_inputs().items()}
        expected = np.asarray(ref.reference(**inputs))
    out = kernel(**inputs)
    diff = out - expected
    print("rel_fro:", np.linalg.norm(diff) / np.linalg.norm(expected))
    print("max_abs:", np.abs(diff).max())
